# revision 1
# baseline (speedup 1.0000x reference)
"""Trainium2 Bass kernel for nn_MixtureOfExperts (argmax-routed SwiGLU MoE).

Strategy (expert-parallel across 8 NeuronCores):
  - Host computes router logits (fp64 matmul, tiny: 4096x1024x8) and the
    argmax expert per token.  Top-2 logit gaps are >=1.7e-4 while fp32
    rounding noise is ~1e-6, so routing is insensitive to arithmetic order.
  - Tokens are grouped by selected expert; each core receives only its
    expert's tokens (padded to a common capacity C) plus that expert's
    gate/up/down banks.  Each core computes the SwiGLU for its tokens only:
        h = silu(x @ gw) * (x @ uw);  y = h @ dw
    This does 1/E of the reference FLOPs (the reference computes all E
    experts densely and discards all but the argmax one).
  - Host scatters per-core outputs back to token positions.

Layout choices: x is shipped pre-transposed as [D, C] so the contraction
dim D lands on SBUF partitions for both matmul operands; mm1 produces
h^T [H, C] tiles which are exactly the stationary operand layout needed
for mm2 (contraction over H), so no on-chip transposes are required.
gate/up weights are host-packed chunk-interleaved so each weight DMA is one
contiguous transfer.

Matmul dtype: float32r (fp32 storage, PE rounds to 11 mantissa bits,
1 cycle/row vs fp32's 4 for free-dim >= 256).  Operands are pre-rounded on
the host with the exact RNE rule of the compiler's fp32_to_fp32r.
"""

import numpy as np

import concourse.mybir as mybir
import concourse.tile as tile
from concourse import bacc
from concourse.bass_utils import run_bass_kernel_spmd

B, T, D, E, H = 4, 1024, 1024, 8, 2048
BT = B * T
NCORES = 8
P = 128
KD = D // P   # k-tiles for mm1 (contraction over D)
KH = H // P   # k-tiles for mm2 (contraction over H)
F32 = mybir.dt.float32
F32R = mybir.dt.float32r

# "fp32"  : exact fp32 matmuls (4 cycles/row on PE)
# "fp32r" : fp32 data, reduced-precision PE mode (1 cycle/row at N>=256)
MM_MODE = "fp32r"

# Interleave gate/up matmul accumulation chains (a-k0, u-k0, a-k1, ...)
INTERLEAVE_GU = True

# gate/up weight chunks over H.  First chunks are small so the critical
# prefix (x + first weight chunk) is minimal before the PE can start.
H_CHUNKS = [(0, 128), (128, 128), (256, 256), (512, 512), (1024, 512),
            (1536, 512)]
assert sum(c for _, c in H_CHUNKS) == H

_BUILD_CACHE = {}


def _chunks(total, step):
    out = []
    o = 0
    while o < total:
        out.append((o, min(step, total - o)))
        o += step
    return out


def _balanced_chunks(total, step):
    """Split `total` into <=step chunks, as equal as possible (32-aligned).

    Keeps every matmul free-dim >= 256 where possible, which float32r needs
    for full-rate (1 cycle/row) operation.
    """
    n = -(-total // step)
    base = -(-total // (n * 32)) * 32
    out = []
    o = 0
    while o < total:
        sz = min(base, total - o)
        out.append((o, sz))
        o += sz
    return out


def round_fp32r(a):
    """Round fp32 array to the float32r grid (RNE at 12 low mantissa bits).

    Bit-exact with neuronxcc's fp32_to_fp32r (verified on random normals +
    subnormal/huge ranges).
    """
    u = np.ascontiguousarray(a, np.float32).view(np.uint32).astype(np.uint64)
    lsb = (u >> 12) & 1
    r = (u + 0x7FF + lsb) & 0xFFFFF000
    return r.astype(np.uint32).view(np.float32)


def _build(C, mm_mode):
    """Build the per-core SPMD Bass kernel for token capacity C."""
    n_chunks = _balanced_chunks(C, 512)   # token tiles in the free dim (mm1)
    m_tiles = _chunks(C, P)               # token tiles in the partition dim (mm2)
    d_chunks = _chunks(D, 512)            # output-column tiles (mm2)

    mdt = F32R if mm_mode == "fp32r" else F32

    nc = bacc.Bacc("TRN2", target_bir_lowering=False, debug=False)
    # xt packed partition-major: [128, KD*C], block k = x^T[k*128:(k+1)*128,:]
    xt = nc.dram_tensor("xt", [P, KD * C], mdt, kind="ExternalInput")
    gup = nc.dram_tensor("gu", [D, 2 * H], mdt, kind="ExternalInput")
    dw = nc.dram_tensor("dw", [H, D], mdt, kind="ExternalInput")
    y = nc.dram_tensor("y", [C, D], F32, kind="ExternalOutput")

    with tile.TileContext(nc) as tc:
        with (
            tc.tile_pool(name="xp", bufs=1) as xp,
            tc.tile_pool(name="hp", bufs=KH) as hp,
            tc.tile_pool(name="w1", bufs=16) as w1,
            tc.tile_pool(name="w2", bufs=KH) as w2,
            tc.tile_pool(name="outp", bufs=3) as outp,
            tc.tile_pool(name="ps", bufs=8, space="PSUM") as ps,
        ):
            # DMA issue costs ~0.65us of serialized sequencer time per
            # dma_start.  Spread issues over several engine queues: at the
            # head every engine is idle, so the 16-DMA critical prefix
            # arms in ~2 rounds instead of 16.
            head_engines = [nc.sync, nc.scalar]
            body_engines = [nc.sync]
            _eng_i = [0]

            def dma(engs, dst, src):
                engs[_eng_i[0] % len(engs)].dma_start(dst, src)
                _eng_i[0] += 1

            # resident activations: one [128, KD*nn] tile per token-chunk
            # (block k at columns [k*nn:(k+1)*nn]), each a single DMA so the
            # critical prefix arms in one issue slot.
            x_t = {}

            def load_x_chunk(ni, engs):
                n0, nn_ = n_chunks[ni]
                t = xp.tile([P, KD * nn_], mdt, tag=f"x{ni}")
                src = xt.rearrange("p (k c) -> p k c", k=KD)[:, :, n0:n0 + nn_]
                dma(engs, t[:].rearrange("p (k c) -> p k c", k=KD), src)
                x_t[ni] = t

            def x_slice(k, ni):
                nn_ = n_chunks[ni][1]
                return x_t[ni][:, k * nn_:(k + 1) * nn_]

            load_x_chunk(0, head_engines)

            # ---- mm1: hT[j] = silu(gw.T x) * (uw.T x), tiled over H ----
            h_t = []
            gu_col = 0
            for ci, (hc0, hcn) in enumerate(H_CHUNKS):
                gut = []
                engs = head_engines if ci <= 1 else body_engines
                for k in range(KD):
                    t = w1.tile([P, 2 * hcn], mdt, tag="w1")
                    dma(engs, t[:],
                        gup[k * P:(k + 1) * P, gu_col:gu_col + 2 * hcn])
                    gut.append(t)
                gu_col += 2 * hcn
                if ci == 0:
                    for ni in range(1, len(n_chunks)):
                        load_x_chunk(ni, body_engines)
                for hs in range(hcn // P):
                    ht = hp.tile([P, C], mdt, tag="h")
                    for ni, (n0, nn_) in enumerate(n_chunks):
                        pa = ps.tile([P, 512], F32, tag="ps", name="pa")[:, :nn_]
                        pu = ps.tile([P, 512], F32, tag="ps", name="pu")[:, :nn_]

                        def mm_a(k):
                            nc.tensor.matmul(
                                pa[:, :],
                                lhsT=gut[k][:, hs * P:(hs + 1) * P],
                                rhs=x_slice(k, ni),
                                start=(k == 0), stop=(k == KD - 1),
                            )

                        def mm_u(k):
                            nc.tensor.matmul(
                                pu[:, :],
                                lhsT=gut[k][:, hcn + hs * P:hcn + (hs + 1) * P],
                                rhs=x_slice(k, ni),
                                start=(k == 0), stop=(k == KD - 1),
                            )

                        if INTERLEAVE_GU:
                            for k in range(KD):
                                mm_a(k)
                                mm_u(k)
                        else:
                            for k in range(KD):
                                mm_a(k)
                            for k in range(KD):
                                mm_u(k)
                        nc.scalar.activation(
                            ht[:, n0:n0 + nn_], pa[:, :],
                            mybir.ActivationFunctionType.Silu,
                        )
                        nc.vector.tensor_mul(
                            ht[:, n0:n0 + nn_], ht[:, n0:n0 + nn_], pu[:, :]
                        )
                    h_t.append(ht)

            # down-proj weights: one [128, D] tile per h k-slice, loaded
            # once and reused by every (nd, m) tile.  Emitted after mm1 so
            # the DMA queue drains mm1's weights first; the scheduler still
            # overlaps these loads with mm1 compute.
            dwt = []
            for k in range(KH):
                t = w2.tile([P, D], mdt, tag="w2")
                dma(body_engines, t[:], dw[k * P:(k + 1) * P, :])
                dwt.append(t)

            # ---- mm2: y = h @ dw, contraction over H ----
            for nd0, ndn in d_chunks:
                for m0, mn in m_tiles:
                    py = ps.tile([P, ndn], F32, tag="ps")
                    for k in range(KH):
                        nc.tensor.matmul(
                            py[:mn, :],
                            lhsT=h_t[k][:, m0:m0 + mn],
                            rhs=dwt[k][:, nd0:nd0 + ndn],
                            start=(k == 0),
                            stop=(k == KH - 1),
                        )
                    ot = outp.tile([P, ndn], F32, tag="out")
                    nc.vector.tensor_copy(ot[:mn, :], py[:mn, :])
                    nc.sync.dma_start(y[m0:m0 + mn, nd0:nd0 + ndn], ot[:mn, :])

    nc.compile()
    return nc


def _get_kernel(C, mm_mode=None):
    """Build (cached).  Falls back to exact fp32 if the f32r build fails."""
    mm_mode = mm_mode or MM_MODE
    key = (C, mm_mode)
    if key not in _BUILD_CACHE:
        try:
            _BUILD_CACHE[key] = (_build(C, mm_mode), mm_mode)
        except Exception:
            if mm_mode == "fp32":
                raise
            _BUILD_CACHE[key] = (_build(C, "fp32"), "fp32")
    return _BUILD_CACHE[key]


def _route(xf, gate_w):
    """argmax expert per token, computed in fp64 on host (negligible work)."""
    logits = xf.astype(np.float64) @ np.asarray(gate_w, np.float64).T
    return logits.argmax(axis=1)


def _pack_gu(gw_e, uw_e):
    """Interleave gate/up banks by H_CHUNKS columns: [D, 2H] with chunk i at
    cumulative offset = [gate chunk | up chunk]."""
    parts = []
    for hc0, hcn in H_CHUNKS:
        parts.append(gw_e[:, hc0:hc0 + hcn])
        parts.append(uw_e[:, hc0:hc0 + hcn])
    return np.ascontiguousarray(np.concatenate(parts, axis=1))


def kernel(x, gate_w, gate_bank, up_bank, down_bank):
    x = np.asarray(x, np.float32)
    assert x.shape == (B, T, D)

    xf = np.ascontiguousarray(x.reshape(BT, D))
    sel = _route(xf, gate_w)
    idx = [np.nonzero(sel == e)[0] for e in range(E)]
    maxc = max(len(i) for i in idx)
    C = max(P, -(-maxc // 32) * 32)

    nc, mode = _get_kernel(C)

    rnd = round_fp32r if mode == "fp32r" else (
        lambda a: np.ascontiguousarray(a, np.float32))
    gate_bank = rnd(gate_bank)
    up_bank = rnd(up_bank)
    down_bank = rnd(down_bank)

    in_maps = []
    for e in range(E):
        xe = np.zeros((D, C), np.float32)
        n = len(idx[e])
        if n:
            xe[:, :n] = rnd(xf[idx[e]].T)
        xe = np.ascontiguousarray(
            xe.reshape(KD, P, C).transpose(1, 0, 2).reshape(P, KD * C))
        in_maps.append({
            "xt": xe,
            "gu": _pack_gu(gate_bank[e], up_bank[e]),
            "dw": np.ascontiguousarray(down_bank[e]),
        })

    res = run_bass_kernel_spmd(nc, in_maps, core_ids=list(range(NCORES)))

    out = np.empty((BT, D), np.float32)
    for e in range(E):
        n = len(idx[e])
        if n:
            out[idx[e]] = res.results[e]["y"][:n]
    return out.reshape(B, T, D)



# revision 5
# speedup vs baseline: 1.0822x; 1.0822x over previous
"""Trainium2 Bass kernel for nn_MixtureOfExperts (argmax-routed SwiGLU MoE).

Strategy (expert-parallel across 8 NeuronCores, bf16 matmuls):
  - Host computes router logits (fp64 matmul, tiny) and the argmax expert
    per token.  Top-2 logit gaps are >=1.7e-4 while fp32 rounding noise is
    ~1e-6, so routing is insensitive to arithmetic order.
  - Each core is assigned one expert and a fixed capacity of C=512 tokens
    (zero-padded).  Tokens beyond 512 for an overloaded expert (a few tens
    out of 4096) are computed on the host in fp32 — this keeps every core
    at exactly 512 tokens (perfect balance, and C=512 means every matmul
    streams full 512-row chunks with no partition-tile waste).
  - Each core computes the SwiGLU for its tokens only:
        h = silu(x @ gw) * (x @ uw);  y = h @ dw
    in bf16 (1 PE cycle/row, same rate as fp32r, half the HBM traffic).
  - Host scatters per-core outputs back to token positions.

Layout: x is shipped pre-transposed and k-major packed ([128, KD*C],
block k = x^T[k*128:(k+1)*128, :]) so the contraction dim D lands on SBUF
partitions; mm1 produces h^T [H, C] tiles which are exactly the stationary
operand layout needed for mm2 (contraction over H).  gate/up weights are
host-packed k-major and chunk-interleaved so each weight chunk is ONE
contiguous DMA (DMA issue costs ~0.6us of sequencer time each; the whole
kernel issues ~20 DMAs instead of ~70).
"""

import numpy as np
import ml_dtypes

import concourse.mybir as mybir
import concourse.tile as tile
from concourse import bacc
from concourse.bass_utils import run_bass_kernel_spmd

B, T, D, E, H = 4, 1024, 1024, 8, 2048
BT = B * T
NCORES = 8
P = 128
KD = D // P   # k-tiles for mm1 (contraction over D)
KH = H // P   # k-tiles for mm2 (contraction over H)
C = 512       # per-core token capacity (matches PSUM bank free size)
F32 = mybir.dt.float32
BF16 = mybir.dt.bfloat16
NPBF16 = ml_dtypes.bfloat16

# gate/up weight chunks over H.  First chunks are small so the critical
# prefix (x + first weight chunk) is minimal before the PE can start.
H_CHUNKS = [(0, 128), (128, 128), (256, 256), (512, 512), (1024, 512),
            (1536, 512)]
assert sum(c for _, c in H_CHUNKS) == H

_BUILD_CACHE = {}

# Optional kwargs forwarded to run_bass_kernel_spmd (test harness sets
# this to enable NTFF tracing; empty for normal use).
RUN_KWARGS = {}
LAST_RESULTS = None


def _build():
    """Build the per-core SPMD Bass kernel (capacity C tokens, bf16)."""
    nc = bacc.Bacc("TRN2", target_bir_lowering=False, debug=False)
    # k-major packed operands: block k of xt is x^T[k*128:(k+1)*128, :C]
    xt = nc.dram_tensor("xt", [P, KD * C], BF16, kind="ExternalInput")
    gu = nc.dram_tensor("gu", [P, KD * 2 * H], BF16, kind="ExternalInput")
    dw = nc.dram_tensor("dw", [P, KH * D], BF16, kind="ExternalInput")
    y = nc.dram_tensor("y", [C, D], F32, kind="ExternalOutput")

    with tile.TileContext(nc) as tc:
        with (
            tc.tile_pool(name="xp", bufs=KD) as xp,
            tc.tile_pool(name="hp", bufs=KH) as hp,
            tc.tile_pool(name="w1s", bufs=3) as w1s,
            tc.tile_pool(name="w1b", bufs=3) as w1b,
            tc.tile_pool(name="w2", bufs=2) as w2,
            tc.tile_pool(name="outp", bufs=3) as outp,
            tc.tile_pool(name="ps", bufs=8, space="PSUM") as ps,
        ):
            # DMA issue costs ~0.6us of serialized sequencer time per
            # dma_start.  At the head every engine is idle, so spread the
            # critical-prefix issues across four engine queues.
            head_engines = [nc.sync, nc.scalar, nc.gpsimd]
            _eng_i = [0]

            def dma(dst, src, engs=head_engines):
                engs[_eng_i[0] % len(engs)].dma_start(dst, src)
                _eng_i[0] += 1

            # x: one [P, C] tile per k-slice, each a single contiguous DMA.
            # The first accumulation chain consumes them in k order, so the
            # PE can start as soon as x[k=0] and the first weight chunk land.
            x_t = []
            for k in range(KD):
                t = xp.tile([P, C], BF16, tag=f"x{k}")
                dma(t[:], xt[:, k * C:(k + 1) * C])
                x_t.append(t)

            # gate/up weights: ONE contiguous DMA per H-chunk (k-major
            # packed on host).  Within chunk ci at column base, slice
            # (k, hs, gate) = [base + k*2hcn + hs*P : +P]
            # (k, hs, up)   = [base + k*2hcn + hcn + hs*P : +P]
            w_t = []
            col = 0
            for ci, (hc0, hcn) in enumerate(H_CHUNKS):
                pool = w1s if hcn <= 256 else w1b
                t = pool.tile([P, KD * 2 * hcn], BF16, tag=pool.name)
                dma(t[:], gu[:, col:col + KD * 2 * hcn])
                w_t.append(t)
                col += KD * 2 * hcn

            # down-proj weights: two DMAs (k-major packed [P, KH*D]).
            dwt = []
            for half in range(2):
                t = w2.tile([P, (KH // 2) * D], BF16, tag="w2")
                dma(t[:], dw[:, half * (KH // 2) * D:(half + 1) * (KH // 2) * D])
                dwt.append(t)

            def dw_slice(k, nd0, ndn):
                t = dwt[k // (KH // 2)]
                base = (k % (KH // 2)) * D
                return t[:, base + nd0:base + nd0 + ndn]

            # ---- mm1: hT[j] = silu(gw.T x) * (uw.T x), tiled over H ----
            h_t = []
            for ci, (hc0, hcn) in enumerate(H_CHUNKS):
                wt = w_t[ci]
                for hs in range(hcn // P):
                    ht = hp.tile([P, C], BF16, tag="h")
                    pa = ps.tile([P, C], F32, tag="ps", name="pa")
                    pu = ps.tile([P, C], F32, tag="ps", name="pu")
                    for k in range(KD):
                        nc.tensor.matmul(
                            pa[:, :],
                            lhsT=wt[:, k * 2 * hcn + hs * P:
                                    k * 2 * hcn + hs * P + P],
                            rhs=x_t[k][:],
                            start=(k == 0), stop=(k == KD - 1),
                        )
                        nc.tensor.matmul(
                            pu[:, :],
                            lhsT=wt[:, k * 2 * hcn + hcn + hs * P:
                                    k * 2 * hcn + hcn + hs * P + P],
                            rhs=x_t[k][:],
                            start=(k == 0), stop=(k == KD - 1),
                        )
                    nc.scalar.activation(
                        ht[:, :], pa[:, :],
                        mybir.ActivationFunctionType.Silu,
                    )
                    nc.vector.tensor_mul(ht[:, :], ht[:, :], pu[:, :])
                    h_t.append(ht)

            # ---- mm2: y = h @ dw, contraction over H ----
            for nd0 in range(0, D, C):
                for m in range(C // P):
                    py = ps.tile([P, C], F32, tag="ps", name="py")
                    for k in range(KH):
                        nc.tensor.matmul(
                            py[:, :],
                            lhsT=h_t[k][:, m * P:(m + 1) * P],
                            rhs=dw_slice(k, nd0, C),
                            start=(k == 0),
                            stop=(k == KH - 1),
                        )
                    ot = outp.tile([P, C], F32, tag="out")
                    nc.vector.tensor_copy(ot[:, :], py[:, :])
                    nc.sync.dma_start(y[m * P:(m + 1) * P, nd0:nd0 + C], ot[:, :])

    nc.compile()
    return nc


def _get_kernel():
    if "k" not in _BUILD_CACHE:
        _BUILD_CACHE["k"] = _build()
    return _BUILD_CACHE["k"]


def _route(xf, gate_w):
    """argmax expert per token, computed in fp64 on host (negligible work)."""
    logits = xf.astype(np.float64) @ np.asarray(gate_w, np.float64).T
    return logits.argmax(axis=1)


def _bf16(a):
    return np.ascontiguousarray(np.asarray(a, np.float32)).astype(NPBF16)


def _pack_gu(gw_e, uw_e):
    """k-major chunk-interleaved [P, KD*2H]: chunk ci holds KD blocks of
    [gate[kP:(k+1)P, hc0:hc0+hcn] | up[...]]."""
    parts = []
    for hc0, hcn in H_CHUNKS:
        for k in range(KD):
            parts.append(gw_e[k * P:(k + 1) * P, hc0:hc0 + hcn])
            parts.append(uw_e[k * P:(k + 1) * P, hc0:hc0 + hcn])
    return np.ascontiguousarray(np.concatenate(parts, axis=1))


def _pack_k_major(a):
    """[R*P, N] -> [P, R*N] with block r = a[r*P:(r+1)*P, :]."""
    r = a.shape[0] // P
    return np.ascontiguousarray(
        a.reshape(r, P, a.shape[1]).transpose(1, 0, 2).reshape(P, -1))


def _silu_swiglu_host(xo, gw, uw, dwn):
    """fp32 reference path for host-computed overflow tokens."""
    a = xo @ gw
    u = xo @ uw
    h = u * (a / (1.0 + np.exp(-a)))
    return h @ dwn


def kernel(x, gate_w, gate_bank, up_bank, down_bank):
    global LAST_RESULTS
    x = np.asarray(x, np.float32)
    assert x.shape == (B, T, D)

    xf = np.ascontiguousarray(x.reshape(BT, D))
    sel = _route(xf, gate_w)
    idx = [np.nonzero(sel == e)[0] for e in range(E)]
    keep = [i[:C] for i in idx]
    over = [i[C:] for i in idx]

    nc = _get_kernel()

    gate_bank = np.asarray(gate_bank, np.float32)
    up_bank = np.asarray(up_bank, np.float32)
    down_bank = np.asarray(down_bank, np.float32)
    gb16 = _bf16(gate_bank)
    ub16 = _bf16(up_bank)
    db16 = _bf16(down_bank)
    x16 = _bf16(xf)

    in_maps = []
    for e in range(E):
        xe = np.zeros((D, C), NPBF16)
        n = len(keep[e])
        if n:
            xe[:, :n] = x16[keep[e]].T
        in_maps.append({
            "xt": _pack_k_major(xe),
            "gu": _pack_gu(gb16[e], ub16[e]),
            "dw": _pack_k_major(db16[e]),
        })

    res = run_bass_kernel_spmd(nc, in_maps, core_ids=list(range(NCORES)),
                               **RUN_KWARGS)
    LAST_RESULTS = res

    out = np.empty((BT, D), np.float32)
    for e in range(E):
        n = len(keep[e])
        if n:
            out[keep[e]] = res.results[e]["y"][:n]
        if len(over[e]):
            out[over[e]] = _silu_swiglu_host(
                xf[over[e]], gate_bank[e], up_bank[e], down_bank[e])
    return out.reshape(B, T, D)


# revision 8
# speedup vs baseline: 1.1212x; 1.0360x over previous
"""Trainium2 Bass kernel for nn_MixtureOfExperts (argmax-routed SwiGLU MoE).

Strategy (expert-parallel across 8 NeuronCores, bf16 matmuls):
  - Host computes router logits (fp64 matmul, tiny) and the argmax expert
    per token.  Top-2 logit gaps are >=1.7e-4 while fp32 rounding noise is
    ~1e-6, so routing is insensitive to arithmetic order.
  - Each core is assigned one expert and a fixed capacity of C=512 tokens
    (zero-padded).  Tokens beyond 512 for an overloaded expert (a few tens
    out of 4096) are computed on the host in fp32 — this keeps every core
    at exactly 512 tokens (perfect balance, and C=512 means every matmul
    streams full 512-row chunks with no partition-tile waste).
  - Each core computes the SwiGLU for its tokens only:
        h = silu(x @ gw) * (x @ uw);  y = h @ dw
    in bf16 (1 PE cycle/row, same rate as fp32r, half the HBM traffic).
  - Host scatters per-core outputs back to token positions.

Layout: x is shipped pre-transposed and k-major packed ([128, KD*C],
block k = x^T[k*128:(k+1)*128, :]) so the contraction dim D lands on SBUF
partitions; mm1 produces h^T [H, C] tiles which are exactly the stationary
operand layout needed for mm2 (contraction over H).  gate/up weights are
host-packed k-major and chunk-interleaved so each weight chunk is ONE
contiguous DMA (DMA issue costs ~0.6us of sequencer time each; the whole
kernel issues ~20 DMAs instead of ~70).
"""

import numpy as np
import ml_dtypes

import concourse.mybir as mybir
import concourse.tile as tile
from concourse import bacc
from concourse.bass_utils import run_bass_kernel_spmd

B, T, D, E, H = 4, 1024, 1024, 8, 2048
BT = B * T
NCORES = 8
P = 128
KD = D // P   # k-tiles for mm1 (contraction over D)
KH = H // P   # k-tiles for mm2 (contraction over H)
C = 512       # per-core token capacity (matches PSUM bank free size)
F32 = mybir.dt.float32
BF16 = mybir.dt.bfloat16
NPBF16 = ml_dtypes.bfloat16

# gate/up weight chunks over H.  First chunks are small so the critical
# prefix (x + first weight chunk) is minimal before the PE can start.
H_CHUNKS = [(0, 128), (128, 128), (256, 256), (512, 512), (1024, 512),
            (1536, 512)]
assert sum(c for _, c in H_CHUNKS) == H

_BUILD_CACHE = {}

# Optional kwargs forwarded to run_bass_kernel_spmd (test harness sets
# this to enable NTFF tracing; empty for normal use).
RUN_KWARGS = {}
LAST_RESULTS = None


def _build():
    """Build the per-core SPMD Bass kernel (capacity C tokens, bf16)."""
    nc = bacc.Bacc("TRN2", target_bir_lowering=False, debug=False)
    # k-major packed operands: block k of xt is x^T[k*128:(k+1)*128, :C]
    xt = nc.dram_tensor("xt", [P, KD * C], BF16, kind="ExternalInput")
    gu = nc.dram_tensor("gu", [P, KD * 2 * H], BF16, kind="ExternalInput")
    dw = nc.dram_tensor("dw", [P, KH * D], BF16, kind="ExternalInput")
    y = nc.dram_tensor("y", [C, D], F32, kind="ExternalOutput")

    with tile.TileContext(nc) as tc:
        with (
            tc.tile_pool(name="xp", bufs=KD) as xp,
            tc.tile_pool(name="hp", bufs=KH) as hp,
            tc.tile_pool(name="w1s", bufs=3) as w1s,
            tc.tile_pool(name="w1b", bufs=3) as w1b,
            tc.tile_pool(name="w2", bufs=2) as w2,
            tc.tile_pool(name="outp", bufs=3) as outp,
            tc.tile_pool(name="ps", bufs=8, space="PSUM") as ps,
        ):
            # DMA issue costs ~0.6us of serialized sequencer time per
            # dma_start.  Only SP (sync) and Activation (scalar) have
            # hardware descriptor generation — gpsimd falls back to slow
            # SWDGE, so keep it out.  At the head both engines' queues are
            # empty; interleave the critical prefix across them.
            _eng_i = [0]

            def dma(dst, src, eng=None):
                engs = [nc.sync, nc.scalar]
                (eng or engs[_eng_i[0] % 2]).dma_start(dst, src)
                _eng_i[0] += 1

            # Critical prefix: the first accumulation chain needs the first
            # gate/up chunk plus the x k-tiles in k order.  scalar issues
            # the first weight chunk (split in half so the PE can start
            # after 256KB); sync starts streaming x k-tiles in parallel.
            hcn0 = H_CHUNKS[0][1]
            w_t = []
            t0 = w1s.tile([P, KD * 2 * hcn0], BF16, tag="w1s")
            nc.scalar.dma_start(t0[:, :KD * hcn0], gu[:, :KD * hcn0])
            nc.scalar.dma_start(t0[:, KD * hcn0:], gu[:, KD * hcn0:KD * 2 * hcn0])
            w_t.append(t0)

            # x: one [P, C] tile per k-slice, each a single contiguous DMA.
            x_t = []
            for k in range(KD):
                t = xp.tile([P, C], BF16, tag=f"x{k}")
                dma(t[:], xt[:, k * C:(k + 1) * C])
                x_t.append(t)

            # remaining gate/up chunks: ONE contiguous DMA per H-chunk
            # (k-major packed on host).  Within chunk ci at column base,
            # slice (k, hs, gate) = [base + k*2hcn + hs*P : +P]
            #       (k, hs, up)   = [base + k*2hcn + hcn + hs*P : +P]
            col = KD * 2 * hcn0
            for ci, (hc0, hcn) in enumerate(H_CHUNKS[1:], start=1):
                pool = w1s if hcn <= 256 else w1b
                t = pool.tile([P, KD * 2 * hcn], BF16, tag=pool.name)
                dma(t[:], gu[:, col:col + KD * 2 * hcn])
                w_t.append(t)
                col += KD * 2 * hcn

            # down-proj weights: two DMAs (k-major packed [P, KH*D]).
            dwt = []
            for half in range(2):
                t = w2.tile([P, (KH // 2) * D], BF16, tag="w2")
                dma(t[:], dw[:, half * (KH // 2) * D:(half + 1) * (KH // 2) * D])
                dwt.append(t)

            def dw_slice(k, nd0, ndn):
                t = dwt[k // (KH // 2)]
                base = (k % (KH // 2)) * D
                return t[:, base + nd0:base + nd0 + ndn]

            # ---- mm1: hT[j] = silu(gw.T x) * (uw.T x), tiled over H ----
            h_t = []
            for ci, (hc0, hcn) in enumerate(H_CHUNKS):
                wt = w_t[ci]
                for hs in range(hcn // P):
                    ht = hp.tile([P, C], BF16, tag="h")
                    pa = ps.tile([P, C], F32, tag="ps", name="pa")
                    pu = ps.tile([P, C], F32, tag="ps", name="pu")
                    for k in range(KD):
                        nc.tensor.matmul(
                            pa[:, :],
                            lhsT=wt[:, k * 2 * hcn + hs * P:
                                    k * 2 * hcn + hs * P + P],
                            rhs=x_t[k][:],
                            start=(k == 0), stop=(k == KD - 1),
                        )
                        nc.tensor.matmul(
                            pu[:, :],
                            lhsT=wt[:, k * 2 * hcn + hcn + hs * P:
                                    k * 2 * hcn + hcn + hs * P + P],
                            rhs=x_t[k][:],
                            start=(k == 0), stop=(k == KD - 1),
                        )
                    nc.scalar.activation(
                        ht[:, :], pa[:, :],
                        mybir.ActivationFunctionType.Silu,
                    )
                    nc.vector.tensor_mul(ht[:, :], ht[:, :], pu[:, :])
                    h_t.append(ht)

            # ---- mm2: y = h @ dw, contraction over H ----
            for nd0 in range(0, D, C):
                for m in range(C // P):
                    py = ps.tile([P, C], F32, tag="ps", name="py")
                    for k in range(KH):
                        nc.tensor.matmul(
                            py[:, :],
                            lhsT=h_t[k][:, m * P:(m + 1) * P],
                            rhs=dw_slice(k, nd0, C),
                            start=(k == 0),
                            stop=(k == KH - 1),
                        )
                    # Drain PSUM in two half-tiles so the DMA of the first
                    # half overlaps the copy of the second (shortens the
                    # critical tail after the last matmul).
                    ot = outp.tile([P, C], F32, tag="out")
                    for h0 in range(0, C, C // 2):
                        nc.vector.tensor_copy(ot[:, h0:h0 + C // 2],
                                              py[:, h0:h0 + C // 2])
                        dma(y[m * P:(m + 1) * P, nd0 + h0:nd0 + h0 + C // 2],
                            ot[:, h0:h0 + C // 2])

    nc.compile()
    return nc


def _get_kernel():
    if "k" not in _BUILD_CACHE:
        _BUILD_CACHE["k"] = _build()
    return _BUILD_CACHE["k"]


def _route(xf, gate_w):
    """argmax expert per token, computed in fp64 on host (negligible work)."""
    logits = xf.astype(np.float64) @ np.asarray(gate_w, np.float64).T
    return logits.argmax(axis=1)


def _bf16(a):
    return np.ascontiguousarray(np.asarray(a, np.float32)).astype(NPBF16)


def _pack_gu(gw_e, uw_e):
    """k-major chunk-interleaved [P, KD*2H]: chunk ci holds KD blocks of
    [gate[kP:(k+1)P, hc0:hc0+hcn] | up[...]]."""
    parts = []
    for hc0, hcn in H_CHUNKS:
        for k in range(KD):
            parts.append(gw_e[k * P:(k + 1) * P, hc0:hc0 + hcn])
            parts.append(uw_e[k * P:(k + 1) * P, hc0:hc0 + hcn])
    return np.ascontiguousarray(np.concatenate(parts, axis=1))


def _pack_k_major(a):
    """[R*P, N] -> [P, R*N] with block r = a[r*P:(r+1)*P, :]."""
    r = a.shape[0] // P
    return np.ascontiguousarray(
        a.reshape(r, P, a.shape[1]).transpose(1, 0, 2).reshape(P, -1))


def _silu_swiglu_host(xo, gw, uw, dwn):
    """fp32 reference path for host-computed overflow tokens."""
    a = xo @ gw
    u = xo @ uw
    h = u * (a / (1.0 + np.exp(-a)))
    return h @ dwn


def kernel(x, gate_w, gate_bank, up_bank, down_bank):
    global LAST_RESULTS
    x = np.asarray(x, np.float32)
    assert x.shape == (B, T, D)

    xf = np.ascontiguousarray(x.reshape(BT, D))
    sel = _route(xf, gate_w)
    idx = [np.nonzero(sel == e)[0] for e in range(E)]
    keep = [i[:C] for i in idx]
    over = [i[C:] for i in idx]

    nc = _get_kernel()

    gate_bank = np.asarray(gate_bank, np.float32)
    up_bank = np.asarray(up_bank, np.float32)
    down_bank = np.asarray(down_bank, np.float32)
    gb16 = _bf16(gate_bank)
    ub16 = _bf16(up_bank)
    db16 = _bf16(down_bank)
    x16 = _bf16(xf)

    in_maps = []
    for e in range(E):
        xe = np.zeros((D, C), NPBF16)
        n = len(keep[e])
        if n:
            xe[:, :n] = x16[keep[e]].T
        in_maps.append({
            "xt": _pack_k_major(xe),
            "gu": _pack_gu(gb16[e], ub16[e]),
            "dw": _pack_k_major(db16[e]),
        })

    res = run_bass_kernel_spmd(nc, in_maps, core_ids=list(range(NCORES)),
                               **RUN_KWARGS)
    LAST_RESULTS = res

    out = np.empty((BT, D), np.float32)
    for e in range(E):
        n = len(keep[e])
        if n:
            out[keep[e]] = res.results[e]["y"][:n]
        if len(over[e]):
            out[over[e]] = _silu_swiglu_host(
                xf[over[e]], gate_bank[e], up_bank[e], down_bank[e])
    return out.reshape(B, T, D)


# revision 11
# speedup vs baseline: 1.1960x; 1.0667x over previous
"""Trainium2 Bass kernel for nn_MixtureOfExperts (argmax-routed SwiGLU MoE).

Strategy (expert-parallel across 8 NeuronCores, bf16 matmuls):
  - Host computes router logits (fp64 matmul, tiny) and the argmax expert
    per token.  Top-2 logit gaps are >=1.7e-4 while fp32 rounding noise is
    ~1e-6, so routing is insensitive to arithmetic order.
  - Each core is assigned one expert and a fixed capacity of C=512 tokens
    (zero-padded).  Tokens beyond 512 for an overloaded expert (a few tens
    out of 4096) are computed on the host in fp32 — this keeps every core
    at exactly 512 tokens (perfect balance, and C=512 means every matmul
    streams full 512-row chunks with no partition-tile waste).
  - Each core computes the SwiGLU for its tokens only:
        h = silu(x @ gw) * (x @ uw);  y = h @ dw
    in bf16 (1 PE cycle/row, same rate as fp32r, half the HBM traffic).
  - Host scatters per-core outputs back to token positions.

Layout: x is shipped pre-transposed and k-major packed ([128, KD*C],
block k = x^T[k*128:(k+1)*128, :]) so the contraction dim D lands on SBUF
partitions; mm1 produces h^T [H, C] tiles which are exactly the stationary
operand layout needed for mm2 (contraction over H).  gate/up weights are
host-packed k-major and chunk-interleaved so each weight chunk is ONE
contiguous DMA (DMA issue costs ~0.6us of sequencer time each; the whole
kernel issues ~20 DMAs instead of ~70).
"""

import numpy as np
import ml_dtypes

import concourse.mybir as mybir
import concourse.tile as tile
from concourse import bacc
from concourse.bass_utils import run_bass_kernel_spmd

B, T, D, E, H = 4, 1024, 1024, 8, 2048
BT = B * T
NCORES = 8
P = 128
KD = D // P   # k-tiles for mm1 (contraction over D)
KH = H // P   # k-tiles for mm2 (contraction over H)
C = 512       # per-core token capacity (matches PSUM bank free size)
F32 = mybir.dt.float32
BF16 = mybir.dt.bfloat16
NPBF16 = ml_dtypes.bfloat16

# gate/up weight chunks over H.  Each chunk is one contiguous DMA; DMA
# queue dispatch is per-partition-row, so rows must be >=4KB (256 H cols
# k-major) to reach full bandwidth — smaller chunks starve the queue.
H_CHUNKS = [(0, 256), (256, 256), (512, 512), (1024, 512), (1536, 512)]
assert sum(c for _, c in H_CHUNKS) == H

_BUILD_CACHE = {}

# Optional kwargs forwarded to run_bass_kernel_spmd (test harness sets
# this to enable NTFF tracing; empty for normal use).
RUN_KWARGS = {}
LAST_RESULTS = None


def _build():
    """Build the per-core SPMD Bass kernel (capacity C tokens, bf16)."""
    nc = bacc.Bacc("TRN2", target_bir_lowering=False, debug=False)
    # k-major packed operands: block k of xt is x^T[k*128:(k+1)*128, :C]
    xt = nc.dram_tensor("xt", [P, KD * C], BF16, kind="ExternalInput")
    gu = nc.dram_tensor("gu", [P, KD * 2 * H], BF16, kind="ExternalInput")
    dw = nc.dram_tensor("dw", [P, KH * D], BF16, kind="ExternalInput")
    y = nc.dram_tensor("y", [C, D], F32, kind="ExternalOutput")

    with tile.TileContext(nc) as tc:
        with (
            tc.tile_pool(name="xp", bufs=1) as xp,
            tc.tile_pool(name="hp", bufs=KH) as hp,
            tc.tile_pool(name="w1s", bufs=3) as w1s,
            tc.tile_pool(name="w1b", bufs=3) as w1b,
            tc.tile_pool(name="w2", bufs=2) as w2,
            tc.tile_pool(name="outp", bufs=3) as outp,
            tc.tile_pool(name="ps", bufs=8, space="PSUM") as ps,
        ):
            # DMA issue costs ~0.6us of serialized sequencer time per
            # dma_start.  Only SP (sync) and Activation (scalar) have
            # hardware descriptor generation — gpsimd falls back to slow
            # SWDGE, so keep it out.  At the head both engines' queues are
            # empty; interleave the critical prefix across them.
            _eng_i = [0]

            def dma(dst, src, eng=None):
                engs = [nc.sync, nc.scalar]
                (eng or engs[_eng_i[0] % 2]).dma_start(dst, src)
                _eng_i[0] += 1

            # Critical prefix: x as ONE [P, KD*C] DMA (8KB rows — DMA queue
            # dispatch is per-partition-row, wide rows reach ~400GB/s while
            # 1KB rows cap near 50GB/s/queue) racing the first gate/up
            # chunk on the other engine's queue.
            xa = xp.tile([P, KD * C], BF16, tag="x")
            nc.sync.dma_start(xa[:], xt[:])
            x_t = [xa[:, k * C:(k + 1) * C] for k in range(KD)]

            # gate/up chunks: ONE contiguous DMA per H-chunk (k-major
            # packed on host).  Within chunk ci at column base,
            # slice (k, hs, gate) = [base + k*2hcn + hs*P : +P]
            #       (k, hs, up)   = [base + k*2hcn + hcn + hs*P : +P]
            w_t = []
            col = 0
            for ci, (hc0, hcn) in enumerate(H_CHUNKS):
                pool = w1s if hcn <= 256 else w1b
                t = pool.tile([P, KD * 2 * hcn], BF16, tag=pool.name)
                dma(t[:], gu[:, col:col + KD * 2 * hcn],
                    eng=nc.scalar if ci == 0 else None)
                w_t.append(t)
                col += KD * 2 * hcn

            # down-proj weights: two DMAs (k-major packed [P, KH*D]).
            dwt = []
            for half in range(2):
                t = w2.tile([P, (KH // 2) * D], BF16, tag="w2")
                dma(t[:], dw[:, half * (KH // 2) * D:(half + 1) * (KH // 2) * D])
                dwt.append(t)

            def dw_slice(k, nd0, ndn):
                t = dwt[k // (KH // 2)]
                base = (k % (KH // 2)) * D
                return t[:, base + nd0:base + nd0 + ndn]

            # ---- mm1: hT[j] = silu(gw.T x) * (uw.T x), tiled over H ----
            h_t = []
            for ci, (hc0, hcn) in enumerate(H_CHUNKS):
                wt = w_t[ci]
                for hs in range(hcn // P):
                    ht = hp.tile([P, C], BF16, tag="h")
                    pa = ps.tile([P, C], F32, tag="ps", name="pa")
                    pu = ps.tile([P, C], F32, tag="ps", name="pu")
                    for k in range(KD):
                        nc.tensor.matmul(
                            pa[:, :],
                            lhsT=wt[:, k * 2 * hcn + hs * P:
                                    k * 2 * hcn + hs * P + P],
                            rhs=x_t[k],
                            start=(k == 0), stop=(k == KD - 1),
                        )
                        nc.tensor.matmul(
                            pu[:, :],
                            lhsT=wt[:, k * 2 * hcn + hcn + hs * P:
                                    k * 2 * hcn + hcn + hs * P + P],
                            rhs=x_t[k],
                            start=(k == 0), stop=(k == KD - 1),
                        )
                    nc.scalar.activation(
                        ht[:, :], pa[:, :],
                        mybir.ActivationFunctionType.Silu,
                    )
                    nc.vector.tensor_mul(ht[:, :], ht[:, :], pu[:, :])
                    h_t.append(ht)

            # ---- mm2: y = h @ dw, contraction over H ----
            for nd0 in range(0, D, C):
                for m in range(C // P):
                    py = ps.tile([P, C], F32, tag="ps", name="py")
                    for k in range(KH):
                        nc.tensor.matmul(
                            py[:, :],
                            lhsT=h_t[k][:, m * P:(m + 1) * P],
                            rhs=dw_slice(k, nd0, C),
                            start=(k == 0),
                            stop=(k == KH - 1),
                        )
                    # Drain PSUM in two half-tiles so the DMA of the first
                    # half overlaps the copy of the second (shortens the
                    # critical tail after the last matmul).
                    ot = outp.tile([P, C], F32, tag="out")
                    for h0 in range(0, C, C // 2):
                        nc.vector.tensor_copy(ot[:, h0:h0 + C // 2],
                                              py[:, h0:h0 + C // 2])
                        dma(y[m * P:(m + 1) * P, nd0 + h0:nd0 + h0 + C // 2],
                            ot[:, h0:h0 + C // 2])

    nc.compile()
    return nc


def _get_kernel():
    if "k" not in _BUILD_CACHE:
        _BUILD_CACHE["k"] = _build()
    return _BUILD_CACHE["k"]


def _route(xf, gate_w):
    """argmax expert per token, computed in fp64 on host (negligible work)."""
    logits = xf.astype(np.float64) @ np.asarray(gate_w, np.float64).T
    return logits.argmax(axis=1)


def _bf16(a):
    return np.ascontiguousarray(np.asarray(a, np.float32)).astype(NPBF16)


def _pack_gu(gw_e, uw_e):
    """k-major chunk-interleaved [P, KD*2H]: chunk ci holds KD blocks of
    [gate[kP:(k+1)P, hc0:hc0+hcn] | up[...]]."""
    parts = []
    for hc0, hcn in H_CHUNKS:
        for k in range(KD):
            parts.append(gw_e[k * P:(k + 1) * P, hc0:hc0 + hcn])
            parts.append(uw_e[k * P:(k + 1) * P, hc0:hc0 + hcn])
    return np.ascontiguousarray(np.concatenate(parts, axis=1))


def _pack_k_major(a):
    """[R*P, N] -> [P, R*N] with block r = a[r*P:(r+1)*P, :]."""
    r = a.shape[0] // P
    return np.ascontiguousarray(
        a.reshape(r, P, a.shape[1]).transpose(1, 0, 2).reshape(P, -1))


def _silu_swiglu_host(xo, gw, uw, dwn):
    """fp32 reference path for host-computed overflow tokens."""
    a = xo @ gw
    u = xo @ uw
    h = u * (a / (1.0 + np.exp(-a)))
    return h @ dwn


def kernel(x, gate_w, gate_bank, up_bank, down_bank):
    global LAST_RESULTS
    x = np.asarray(x, np.float32)
    assert x.shape == (B, T, D)

    xf = np.ascontiguousarray(x.reshape(BT, D))
    sel = _route(xf, gate_w)
    idx = [np.nonzero(sel == e)[0] for e in range(E)]
    keep = [i[:C] for i in idx]
    over = [i[C:] for i in idx]

    nc = _get_kernel()

    gate_bank = np.asarray(gate_bank, np.float32)
    up_bank = np.asarray(up_bank, np.float32)
    down_bank = np.asarray(down_bank, np.float32)
    gb16 = _bf16(gate_bank)
    ub16 = _bf16(up_bank)
    db16 = _bf16(down_bank)
    x16 = _bf16(xf)

    in_maps = []
    for e in range(E):
        xe = np.zeros((D, C), NPBF16)
        n = len(keep[e])
        if n:
            xe[:, :n] = x16[keep[e]].T
        in_maps.append({
            "xt": _pack_k_major(xe),
            "gu": _pack_gu(gb16[e], ub16[e]),
            "dw": _pack_k_major(db16[e]),
        })

    res = run_bass_kernel_spmd(nc, in_maps, core_ids=list(range(NCORES)),
                               **RUN_KWARGS)
    LAST_RESULTS = res

    out = np.empty((BT, D), np.float32)
    for e in range(E):
        n = len(keep[e])
        if n:
            out[keep[e]] = res.results[e]["y"][:n]
        if len(over[e]):
            out[over[e]] = _silu_swiglu_host(
                xf[over[e]], gate_bank[e], up_bank[e], down_bank[e])
    return out.reshape(B, T, D)
